# revision 12
# baseline (speedup 1.0000x reference)
"""CMC@k accuracy kernel for Trainium2 (8 NeuronCores, SPMD).

Algorithm (per flank of G=8192 rows, D=256, k=5):
  reference = mean over rows of [any of the k nearest neighbours (excl. self)
  shares the row's label].

Reformulation that avoids argsort: for row i let
    score[i,j] = sq[j] - 2*dot[i,j]        (= dist[i,j] - sq[i], same ordering)
    dm[i]      = min over same-label j!=i of score[i,j]
    cnt[i]     = #{ j : score[i,j] < dm[i] }   (includes self, strict <)
  match[i] <=> 1 <= cnt[i] <= k.
If the row's label is unique, dm is huge and cnt=G > k -> no match, which is
what the reference computes too.

Host-side marshalling: each flank is sorted by label (metric is permutation
invariant), so same-label points are contiguous and the masked min only needs
a narrow column window around the diagonal.  Each of the 4 cores per flank
gets the sorted flank *rotated* so its own 2048 query rows sit at local rows
0..2047 — this keeps the SPMD program identical across cores; the wrapped
label-run at the rotation cut is handled by an extra window segment at the
array tail for slab 0.

Device per slab of 128 query rows:
  PE:  psum[i,jc] = dot(e_i, e_j) - 0.5*sq[j]   (2x K=128 matmuls + 1x K=1)
  ACT: score = -2 * psum  (PSUM->SBUF evacuation, func=Copy scale=-2)
  DVE: neBIG = (lab_win != lab_i) * 1e6 (+1e6 on the self diagonal)
       dm    = min(score_win + neBIG)            (tensor_tensor_reduce)
       cnt   = sum(score < dm)                   (tensor_scalar accum)
       match = (cnt <= k)
Final: per-core match count -> [1,1] output; host sums and divides.
"""
import os
import sys
import numpy as np

sys.path.insert(0, "/opt/trn_rl_repo")

NUM_FLANKS = 2
N, D = 16384, 256
G = N // NUM_FLANKS            # 8192 rows per flank
NCORES = 8
CORES_PER_FLANK = NCORES // NUM_FLANKS
Q = G // CORES_PER_FLANK       # 2048 query rows per core
NSLABS = Q // 128              # 16 slabs per core
M = 64                         # window margin (>= max same-label run)
W = 128 + 2 * M                # window width
BIG = 1.0e6
CHUNK = 512                    # matmul free dim (one PSUM bank, fp32)
PTILE = 2048                   # evacuation granularity (4 PSUM banks)

_cached = {}


def _build_program(k: int):
    import concourse.bass as bass
    import concourse.bacc as bacc
    import concourse.tile as tile
    from concourse import mybir

    f32 = mybir.dt.float32
    bf16 = mybir.dt.bfloat16
    Alu = mybir.AluOpType
    Act = mybir.ActivationFunctionType

    nc = bacc.Bacc()
    emb_d = nc.dram_tensor("emb", [G, D], f32, kind="ExternalInput")
    labf_d = nc.dram_tensor("labf", [G], f32, kind="ExternalInput")
    ident_d = nc.dram_tensor("ident", [128, 128], f32, kind="ExternalInput")
    out_d = nc.dram_tensor("out", [1, 1], f32, kind="ExternalOutput")

    with tile.TileContext(nc) as tc:
        with tc.tile_pool(name="singles", bufs=1) as singles:
            # ---------------- constants / label tiles ----------------
            ident = singles.tile([128, 128], f32)
            nc.gpsimd.dma_start(ident[:], ident_d[:])
            diag_big = singles.tile([128, 128], f32)

            ones_k1 = singles.tile([1, 128], f32)
            nc.vector.memset(ones_k1[:], 1.0)
            ones_col = singles.tile([128, 1], f32)
            nc.vector.memset(ones_col[:], 1.0)

            # labb: broadcast labels over partitions; layout:
            #   cols [0,M)           <- labf[G-M:G]   (wrapped tail)
            #   cols [M, M+Q+M)      <- labf[0:Q+M]
            labb_raw = singles.tile([128, 2 * M + Q], f32)
            nc.gpsimd.dma_start(
                labb_raw[:, 0:M], labf_d[G - M:G].partition_broadcast(128)
            )
            nc.gpsimd.dma_start(
                labb_raw[:, M:], labf_d[0:Q + M].partition_broadcast(128)
            )
            # labiT[i, t] = labf[128 t + i]  (per-slab query labels)
            labiT_raw = singles.tile([128, NSLABS], f32)
            nc.gpsimd.dma_start(
                labiT_raw[:], labf_d[0:Q].rearrange("(t p) -> p t", p=128)
            )
            # DVE-owned copies: each copy absorbs one DMA-lane wait so later
            # DVE consumers carry no cross-lane waits.
            labb = singles.tile([128, 2 * M + Q], f32)
            nc.vector.tensor_copy(labb[:, 0:M], labb_raw[:, 0:M])
            nc.vector.tensor_copy(labb[:, M:], labb_raw[:, M:])
            labiT = singles.tile([128, NSLABS], f32)
            nc.vector.tensor_copy(labiT[:], labiT_raw[:])

            # ---------------- build eT = emb.T in SBUF ----------------
            eT = [
                singles.tile([128, G], f32, tag=f"eT{h}", name=f"eT{h}")
                for h in range(2)
            ]
            with (
                tc.tile_pool(name="stage", bufs=6) as stage,
                tc.tile_pool(name="tp", bufs=4, space="PSUM") as tp,
            ):
                # first PE op: absorbs the ident DMA wait and produces
                # diag_big (= BIG * I, since I.T == I) via the ACT evac.
                pt0 = tp.tile([128, 512], f32, tag="tp")
                nc.tensor.transpose(pt0[:, 0:128], ident[:], ident[:])
                nc.scalar.activation(
                    diag_big[:], pt0[:, 0:128], Act.Copy, scale=BIG
                )
                # groups of 4 row-tiles share one PSUM bank per d-half
                for g in range(G // 512):
                    etiles = []
                    for u in range(4):
                        jt = 4 * g + u
                        et = stage.tile([128, D], f32, tag="stage")
                        nc.gpsimd.dma_start(et[:], emb_d[128 * jt:128 * (jt + 1), :])
                        etiles.append(et)
                    for h in range(2):
                        pt = tp.tile([128, 512], f32, tag="tp")
                        for u in range(4):
                            nc.tensor.transpose(
                                pt[:, 128 * u:128 * (u + 1)],
                                etiles[u][:, 128 * h:128 * (h + 1)],
                                ident[:],
                            )
                        nc.scalar.activation(
                            eT[h][:, 512 * g:512 * (g + 1)], pt[:], Act.Copy
                        )

            # ---------------- sqbm_row[0, j] = -0.5 * sq[j] ----------------
            # sq[j] = sum_d eT[d, j]^2 via ones-matmul over squared eT
            sqbm_row = singles.tile([1, G], f32)
            with (
                tc.tile_pool(name="esq", bufs=1) as esqp,
                tc.tile_pool(name="sqp", bufs=2, space="PSUM") as sqp,
            ):
                esq = [
                    esqp.tile([128, G], f32, tag=f"esq{h}", name=f"esq{h}")
                    for h in range(2)
                ]
                for h in range(2):
                    nc.vector.tensor_tensor(
                        out=esq[h][:], in0=eT[h][:], in1=eT[h][:], op=Alu.mult
                    )
                for q in range(4):
                    pq = sqp.tile([1, PTILE], f32, tag="sq")
                    for c in range(PTILE // CHUNK):
                        cols = slice(PTILE * q + CHUNK * c, PTILE * q + CHUNK * (c + 1))
                        for h in range(2):
                            nc.tensor.matmul(
                                pq[:, CHUNK * c:CHUNK * (c + 1)],
                                ones_col[:],
                                esq[h][:, cols],
                                start=(h == 0),
                                stop=(h == 1),
                            )
                    nc.scalar.activation(
                        sqbm_row[:, PTILE * q:PTILE * (q + 1)], pq[:],
                        Act.Copy, scale=-0.5,
                    )

            # ---------------- main loop over 16 slabs ----------------
            match_acc = singles.tile([128, NSLABS], f32)

            with (
                tc.tile_pool(name="scores", bufs=2) as scores,
                tc.tile_pool(name="small", bufs=2) as small,
                tc.tile_pool(name="mm", bufs=2, space="PSUM") as mmp,
            ):
                for t in range(NSLABS):
                    score = scores.tile([128, G], f32, tag="score")
                    lhs0 = eT[0][:, 128 * t:128 * (t + 1)]
                    lhs1 = eT[1][:, 128 * t:128 * (t + 1)]
                    for q in range(G // PTILE):
                        pm = mmp.tile([128, PTILE], f32, tag="mm")
                        for c in range(PTILE // CHUNK):
                            pslice = pm[:, CHUNK * c:CHUNK * (c + 1)]
                            cols = slice(
                                PTILE * q + CHUNK * c, PTILE * q + CHUNK * (c + 1)
                            )
                            nc.tensor.matmul(
                                pslice, lhs0, eT[0][:, cols], start=True, stop=False
                            )
                            nc.tensor.matmul(
                                pslice, lhs1, eT[1][:, cols], start=False, stop=False
                            )
                            nc.tensor.matmul(
                                pslice,
                                ones_k1[:, 0:128],
                                sqbm_row[:, cols],
                                start=False,
                                stop=True,
                            )
                        nc.scalar.activation(
                            score[:, PTILE * q:PTILE * (q + 1)],
                            pm[:],
                            Act.Copy,
                            scale=-2.0,
                        )

                    # ---- windowed masked min -> dm ----
                    lab_i = labiT[:, t:t + 1]
                    dm = small.tile([128, 1], f32, tag="dm")
                    ne = small.tile([128, W], f32, tag="ne")
                    nc.vector.tensor_scalar(
                        ne[:], labb[:, 128 * t:128 * t + W], lab_i, BIG,
                        op0=Alu.not_equal, op1=Alu.mult,
                    )
                    nc.vector.tensor_tensor(
                        out=ne[:, M:M + 128], in0=ne[:, M:M + 128],
                        in1=diag_big[:], op=Alu.add,
                    )
                    mw = small.tile([128, W], f32, tag="mw")
                    if t == 0:
                        # wrapped tail: score cols [G-M, G) sit at labb[:, 0:M]
                        nc.vector.tensor_tensor(
                            out=mw[:, 0:M], in0=score[:, G - M:G],
                            in1=ne[:, 0:M], op=Alu.add,
                        )
                        nc.vector.tensor_tensor(
                            out=mw[:, M:W], in0=score[:, 0:128 + M],
                            in1=ne[:, M:W], op=Alu.add,
                        )
                    else:
                        lo = 128 * t - M
                        nc.vector.tensor_tensor(
                            out=mw[:], in0=score[:, lo:lo + W], in1=ne[:],
                            op=Alu.add,
                        )
                    nc.vector.tensor_reduce(
                        dm[:], mw[:], axis=mybir.AxisListType.X, op=Alu.min
                    )

                    # ---- count strictly-smaller scores ----
                    cnt = small.tile([128, 1], f32, tag="cnt")
                    nc.vector.tensor_scalar(
                        score[:], score[:], dm[:], None,
                        op0=Alu.is_lt, op1=Alu.add, accum_out=cnt[:],
                    )
                    nc.vector.tensor_scalar(
                        match_acc[:, t:t + 1], cnt[:], float(k), None,
                        op0=Alu.is_le,
                    )

            # ---------------- final reduction ----------------
            msum = singles.tile([128, 1], f32)
            nc.vector.reduce_sum(msum[:], match_acc[:], axis=mybir.AxisListType.X)
            with tc.tile_pool(name="fin", bufs=1, space="PSUM") as finp:
                pf = finp.tile([1, 1], f32)
                nc.tensor.matmul(pf[:], ones_col[:], msum[:], start=True, stop=True)
                osb = singles.tile([1, 1], f32)
                nc.scalar.activation(osb[:], pf[:], Act.Copy)
                nc.gpsimd.dma_start(out_d[:], osb[:])

    nc.finalize()
    return nc


def _prepare_inputs(embeddings, labels):
    """Sort each flank by label, build per-core rotated inputs."""
    emb = np.ascontiguousarray(np.asarray(embeddings, dtype=np.float32))
    lab = np.asarray(labels)
    ident = np.eye(128, dtype=np.float32)
    in_maps = []
    for f in range(NUM_FLANKS):
        ef = emb[f * G:(f + 1) * G]
        lf = lab[f * G:(f + 1) * G]
        order = np.argsort(lf, kind="stable")
        ef, lf = ef[order], lf[order]
        # window-margin safety: same-label runs must fit in M
        runs = np.diff(
            np.flatnonzero(np.concatenate(([True], lf[1:] != lf[:-1], [True])))
        )
        assert runs.max() <= M, f"label run {runs.max()} exceeds window margin {M}"
        lf32 = lf.astype(np.float32)
        for cc in range(CORES_PER_FLANK):
            r = Q * cc
            in_maps.append({
                "emb": np.ascontiguousarray(np.roll(ef, -r, axis=0)),
                "labf": np.ascontiguousarray(np.roll(lf32, -r)),
                "ident": ident,
            })
    return in_maps


def kernel(embeddings, labels, flanks, k):
    from concourse.bass_utils import run_bass_kernel_spmd

    k = int(k)
    if "nc" not in _cached:
        _cached["nc"] = _build_program(k)
    nc = _cached["nc"]
    in_maps = _prepare_inputs(embeddings, labels)
    res = run_bass_kernel_spmd(nc, in_maps, list(range(NCORES)))
    total = sum(float(r["out"][0, 0]) for r in res.results)
    return np.float32(total / N)


if __name__ == "__main__":
    sys.path.insert(0, os.path.dirname(os.path.abspath(__file__)))
    from reference import setup_inputs, reference

    inputs = setup_inputs()
    expected = float(reference(**inputs))
    got = float(kernel(**{kk: np.asarray(v) for kk, v in inputs.items()}))
    rel = abs(got - expected) / abs(expected)
    print(f"expected={expected} got={got} rel={rel:.3e}")
